# revision 39
# baseline (speedup 1.0000x reference)
"""Sharded Bass kernel for nn_AggrGATGated: gated GNN message passing.

Sharding: nodes are sharded across the 8 cores. Each edge's gather index ==
its scatter index (the reference gathers src_gated[edge_idx] and scatter-adds
to the same edge_idx), so a core that owns a node range processes exactly the
edges targeting it: NO collectives are needed at all.

Within a core, nodes are re-packed into NB blocks of 128 PSUM slots by a
worst-fit-decreasing bin packer so that each (block, edge-type) holds <= 128
edges; this makes every (block, type) exactly one 128-edge matmul tile.
The host precomputes, per core: transposed 128x128 edge-feature tiles
(chunk-packed for large DMAs) and BOTH orientations of each tile's one-hot
edge->slot matrix in fp8 (exact for 0/1 values; fp8 stationary x bf16 moving
matmuls are legal on TRN2).

Real-hardware cost structure (measured on-device, which diverges from the
TimelineSim cost model): contiguous-rhs bf16 matmuls stream ~2 cols/cycle,
strided rhs APs halve the rate, K=1 matmuls force a PE tile_size reconfig,
and per-queue DMA caps at ~140-165 GB/s. The kernel therefore uses three
contiguous matmuls per tile via a per-tile [gate|val] PSUM bank:
  mm1: bank[e, 0:2U]  = efT.T @ [W_gate_e[t] | W_dense[t]]   (N=512, start)
  mm2: bank[e, 0:U]  += ohT.T @ sg_block_b                   (N=256, stop)
  gate = sigmoid(bank[:, 0:U]) -> bf16                  (ACT)
  valb = bank[:, U:2U] + btab_t -> bf16                 (DVE; bias off-PE)
  msgs = gate * valb                                    (DVE, bf16 2x mode)
  mm3: pso[b%2] += oh.T @ msgs    (N=256, PSUM-accumulated, LAG-deferred)
Per 2-block group: one f32->bf16 flush copy (ACT) + one grouped DMA store;
output is bf16, upcast to f32 on host.

Phase 1 fills the SBUF-resident sg table (sg = features_shard @ W_gate, two
blocks per PSUM bank, copies alternating ACT/DVE) and is emitted interleaved
into the phase-2 tile stream (P1_AHEAD blocks ahead) so its featT DMA and
matmuls overlap phase-2 compute instead of serializing in the in-order PE
queue. DMA queues: eft + out stores on SP, featT + oh on ACT(HWDGE), ohT on
the GPSIMD SWDGE ring as a third channel.
"""
import dataclasses
import numpy as np
import ml_dtypes

def _bf(x):
    return np.asarray(x).astype(ml_dtypes.bfloat16)

def _f8(x):
    return np.asarray(x).astype(ml_dtypes.float8_e4m3)

import concourse.bass as bass
import concourse.bacc as bacc
import concourse.mybir as mybir
from concourse.tile import TileContext

F32 = mybir.dt.float32
I32 = mybir.dt.int32
BF16 = mybir.dt.bfloat16
FP8 = mybir.dt.float8e4
AF = mybir.ActivationFunctionType
ALU = mybir.AluOpType


@dataclasses.dataclass
class Cfg:
    ncores: int = 8
    R: int = 12544          # real node coverage per core (ceil(BN/8) to 128)
    NB: int = 132           # device blocks per core (>= R/128; slack for packing)
    F: int = 256            # node feature dim
    U: int = 256            # output dim
    FE: int = 128           # edge feature dim
    T: int = 3              # edge types
    BN: int = 100000        # real node count (B*N)
    GCH: int = 16           # eft/oh chunk, in tiles
    JB: int = 8             # featT/sg blocks per DMA group
    OB: int = 2             # out blocks per psum group / DMA store

    @property
    def NBLK(self):
        return self.NB

    @property
    def RS(self):
        return self.NB * 128    # device node slots per core


def _pack_core(d: np.ndarray, NB: int, cap: int = 128):
    """Assign nodes (degree vectors d [Rn, T]) to NB blocks of <=128 slots with
    per-type edge-count <= cap. Worst-fit decreasing; overflows allowed (they
    just bump the tile count). Returns assign [Rn]."""
    Rn, T = d.shape
    order = np.argsort(-d.sum(axis=1), kind='stable')
    rem = np.full((NB, T), cap, np.int64)
    slots = np.full(NB, 128, np.int64)
    assign = np.empty(Rn, np.int64)
    for n in order:
        dn = d[n]
        fits = (rem >= dn).all(axis=1) & (slots > 0)
        if fits.any():
            score = (rem - dn).min(axis=1).astype(np.float64)
            score[~fits] = -np.inf
            b = int(np.argmax(score * 128 + slots))
        else:
            ok = slots > 0
            over = np.maximum(dn - rem, 0).sum(axis=1).astype(np.float64)
            over[~ok] = np.inf
            b = int(np.argmin(over))
        assign[n] = b
        rem[b] -= dn
        slots[b] -= 1
    return assign


def preprocess(cfg: Cfg, edge_idx: np.ndarray, edge_feats: np.ndarray):
    """Pack nodes into blocks, bucket edges per (block, type) tile.

    Returns (K, NT, per_core, slot_of_node) where per_core holds the device
    input arrays and slot_of_node [NC, R] maps local node -> device slot."""
    NC, R, NB, T, FE, GCH = cfg.ncores, cfg.R, cfg.NB, cfg.T, cfg.FE, cfg.GCH
    edge_idx = np.asarray(edge_idx)

    # per-node type degrees over the padded node space
    deg = np.zeros((NC * R, T), np.int32)
    for t in range(T):
        deg[:, t] = np.bincount(edge_idx[t], minlength=NC * R)[:NC * R]

    slot_of_node = np.zeros((NC, R), np.int64)
    for c in range(NC):
        assign = _pack_core(deg[c * R:(c + 1) * R], NB)
        order = np.argsort(assign, kind='stable')
        ranks = np.empty(R, np.int64)
        # rank within block
        blocksorted = assign[order]
        start = np.searchsorted(blocksorted, np.arange(NB))
        pos = np.arange(R) - start[blocksorted]
        ranks[order] = pos
        slot_of_node[c] = assign * 128 + ranks

    # per (core, block, type) counts using slots
    counts = np.zeros((NC, NB, T), np.int64)
    eslots = []          # per t: (sorted edge ids, their cores, their slots)
    for t in range(T):
        idx = edge_idx[t]
        core = idx // R
        loc = idx - core * R
        slot = slot_of_node[core, loc]
        key = core * (NB * 128) + slot
        o = np.argsort(key, kind='stable')
        eslots.append((o, core[o], slot[o]))
        blk = core[o] * NB + (slot[o] >> 7)
        cnt = np.bincount(blk, minlength=NC * NB)
        counts[:, :, t] = cnt.reshape(NC, NB)

    K = -(-counts.max(axis=0) // 128)        # [NB, T], may contain 0
    NT = int(K.sum())
    NCH = -(-NT // GCH)
    Kcum = np.zeros((NB, T), np.int64)
    acc = 0
    for b in range(NB):
        for t in range(T):
            Kcum[b, t] = acc
            acc += int(K[b, t])

    per_core = []
    for c in range(NC):
        ids = np.full((NT, 128), -1, dtype=np.int64)
        offs = np.full((NT, 128), -1, dtype=np.int64)
        for t in range(T):
            o, ecore, eslot = eslots[t]
            lo = np.searchsorted(ecore, c)
            hi = np.searchsorted(ecore, c + 1)
            sl = eslot[lo:hi]
            eid = o[lo:hi]
            bounds = np.searchsorted(sl, np.arange(NB + 1) * 128)
            for b in range(NB):
                s, e = bounds[b], bounds[b + 1]
                n = e - s
                if n == 0:
                    continue
                ti = int(Kcum[b, t])
                for k in range(int(K[b, t])):
                    a0, a1 = k * 128, min((k + 1) * 128, n)
                    m = a1 - a0
                    if m <= 0:
                        break
                    ids[ti + k, :m] = eid[s + a0:s + a1]
                    offs[ti + k, :m] = sl[s + a0:s + a1] & 127
        # eft tiles (transposed), chunk-major packing
        type_of_tile = np.zeros(NT, np.int64)
        for b in range(NB):
            for t in range(T):
                ti = int(Kcum[b, t])
                type_of_tile[ti:ti + int(K[b, t])] = t
        eft = np.zeros((NT, 128, FE), dtype=np.float32)
        for t in range(T):
            sel = np.nonzero(type_of_tile == t)[0]
            idsf = ids[sel]
            v = idsf >= 0
            ef = np.zeros((len(sel), 128, FE), np.float32)
            ef[v] = np.asarray(edge_feats[t])[idsf[v]]
            eft[sel] = ef
        eftT = eft.transpose(0, 2, 1)
        eftC = np.zeros((NCH, FE, GCH * 128), ml_dtypes.bfloat16)
        for ch in range(NCH):
            n_t = min(GCH, NT - ch * GCH)
            blk = eftT[ch * GCH: ch * GCH + n_t]
            eftC[ch, :, :n_t * 128] = blk.transpose(1, 0, 2).reshape(FE, n_t * 128)
        # one-hot (edge->slot) tiles in fp8, both orientations, chunk-packed
        oh = np.zeros((NT, 128, 128), np.float32)   # [tile, edge, slot]
        tt, ee = np.nonzero(offs >= 0)
        oh[tt, ee, offs[tt, ee]] = 1.0
        ohT = oh.transpose(0, 2, 1)                 # [tile, slot, edge]
        ohC = np.zeros((NCH, 128, GCH * 128), ml_dtypes.float8_e4m3)
        ohTC = np.zeros((NCH, 128, GCH * 128), ml_dtypes.float8_e4m3)
        for ch in range(NCH):
            n_t = min(GCH, NT - ch * GCH)
            blk = oh[ch * GCH: ch * GCH + n_t]
            ohC[ch, :, :n_t * 128] = _f8(
                blk.transpose(1, 0, 2).reshape(128, n_t * 128))
            blkT = ohT[ch * GCH: ch * GCH + n_t]
            ohTC[ch, :, :n_t * 128] = _f8(
                blkT.transpose(1, 0, 2).reshape(128, n_t * 128))
        per_core.append(dict(eft=eftC, ohc=ohC, ohtc=ohTC))
    return K, NT, per_core, slot_of_node


def make_feat_inputs(cfg: Cfg, features: np.ndarray, slot_of_node: np.ndarray):
    """Per-core packed featT over device slots: [NBJ, 128, JB*FKC*128]."""
    NC, R, F, JB, NB = cfg.ncores, cfg.R, cfg.F, cfg.JB, cfg.NB
    FKC = F // 128
    RS = cfg.RS
    NBJ = -(-NB // JB)
    feat_flat = np.asarray(features).reshape(-1, F)
    outs = []
    for c in range(NC):
        fs = np.zeros((RS, F), np.float32)
        lo, hi = c * R, min((c + 1) * R, feat_flat.shape[0])
        if hi > lo:
            fs[slot_of_node[c][:hi - lo]] = feat_flat[lo:hi]
        fc = fs.reshape(NB, 128, FKC, 128)
        ft = fc.transpose(0, 2, 3, 1)                # [NB, FKC, f, n]
        packed = np.zeros((NBJ, 128, JB * FKC * 128), ml_dtypes.bfloat16)
        for jc in range(NBJ):
            nb = min(JB, NB - jc * JB)
            blk = ft[jc * JB: jc * JB + nb]
            packed[jc, :, :nb * FKC * 128] = (
                blk.transpose(2, 0, 1, 3).reshape(128, nb * FKC * 128))
        outs.append(packed)
    return outs


def build_kernel(cfg: Cfg, K: np.ndarray, NT: int, dbg: bool = False, bench_iters: int = 0, ablate: str = ''):
    NBLK, T, U, FE, F = cfg.NBLK, cfg.T, cfg.U, cfg.FE, cfg.F
    GCH, JB, OB = cfg.GCH, cfg.JB, cfg.OB
    FKC = F // 128
    NCH = -(-NT // GCH)
    NBJ = -(-NBLK // JB)

    nc = bacc.Bacc("TRN2", target_bir_lowering=False, debug=False,
                   num_devices=cfg.ncores)

    featT = nc.dram_tensor("featT", [NBJ, 128, JB * FKC * 128], BF16,
                           kind="ExternalInput")
    wg = nc.dram_tensor("wg", [FKC, 128, U], BF16, kind="ExternalInput")
    wcat = nc.dram_tensor("wcat", [T, FE, 2 * U], BF16, kind="ExternalInput")
    btab = nc.dram_tensor("btab", [128, T * U], BF16, kind="ExternalInput")
    eft = nc.dram_tensor("eft", [NCH, FE, GCH * 128], BF16, kind="ExternalInput")
    ohc = nc.dram_tensor("ohc", [NCH, 128, GCH * 128], FP8, kind="ExternalInput")
    ohtc = nc.dram_tensor("ohtc", [NCH, 128, GCH * 128], FP8,
                          kind="ExternalInput")
    out = nc.dram_tensor("out", [NBLK, 128, U], BF16, kind="ExternalOutput")

    with TileContext(nc) as tc:
        with (
            tc.tile_pool(name="const", bufs=1) as constp,
            tc.tile_pool(name="ftile", bufs=3) as ftp,
            tc.tile_pool(name="eftl", bufs=6) as eftp,
            tc.tile_pool(name="ohl", bufs=6) as ohp,
            tc.tile_pool(name="ohtl", bufs=6) as ohtp,
            tc.tile_pool(name="gate", bufs=10) as gatep,
            tc.tile_pool(name="valb", bufs=10) as valbp,
            tc.tile_pool(name="msgs", bufs=12) as msgsp,
            tc.tile_pool(name="outst", bufs=2) as outstp,
            tc.tile_pool(name="bank", bufs=6, space="PSUM") as bankp,
            tc.tile_pool(name="psout", bufs=2, space="PSUM") as psoutp,
        ):
            # ---- constants ----
            wg_sb = []
            for kc in range(FKC):
                w = constp.tile([128, U], BF16, tag=f"wg{kc}")
                nc.scalar.dma_start(out=w[:, :], in_=wg[kc, :, :])
                wg_sb.append(w)
            wcat_sb = []
            for t in range(T):
                w = constp.tile([FE, 2 * U], BF16, tag=f"wcat{t}")
                nc.scalar.dma_start(out=w[:, :], in_=wcat[t, :, :])
                wcat_sb.append(w)
            btab_sb = constp.tile([128, T * U], BF16, tag="btab")
            nc.scalar.dma_start(out=btab_sb[:, :], in_=btab[:, :])

            sgtab = constp.tile([128, NBLK * U], BF16, tag="sgtab")

            # ---- phase 1 emitters (interleaved into the phase-2 stream) ----
            import contextlib
            loop_cm = (tc.For_i(0, bench_iters, 1, hint_engines=(
                mybir.EngineType.PE, mybir.EngineType.DVE,
                mybir.EngineType.Activation, mybir.EngineType.Pool,
                mybir.EngineType.SP))
                if bench_iters else contextlib.nullcontext())
            loop_ctx = loop_cm.__enter__() if bench_iters else None

            ft_tiles = {}

            def ensure_ft(jc):
                if jc in ft_tiles or jc >= NBJ:
                    return
                nb = min(JB, NBLK - jc * JB)
                ft = ftp.tile([128, JB * FKC * 128], BF16, tag="ft",
                              name=f"ft{jc}")
                nc.scalar.dma_start(out=ft[:, :nb * FKC * 128],
                                    in_=featT[jc, :, :nb * FKC * 128])
                ft_tiles[jc] = ft

            def emit_p1_pair(j0):
                """sg for blocks j0, j0+1 (paired in one PSUM bank)."""
                jc = j0 // JB
                ensure_ft(jc)
                ensure_ft(jc + 1)
                ft = ft_tiles[jc]
                jj = j0 - jc * JB
                npr = min(2, NBLK - j0)
                ps = bankp.tile([128, 2 * U], F32, tag="bk", name=f"p1_{j0}")
                for d in range(npr):
                    for kc in range(FKC):
                        o = ((jj + d) * FKC + kc) * 128
                        nc.tensor.matmul(ps[:, d * U:(d + 1) * U],
                                         ft[:, o:o + 128], wg_sb[kc][:, :],
                                         start=(kc == 0),
                                         stop=(kc == FKC - 1),
                                         skip_group_check=True)
                c0 = j0
                if (j0 // 2) % 2 == 0:
                    nc.scalar.copy(sgtab[:, c0 * U:(c0 + npr) * U],
                                   ps[:, :npr * U])
                else:
                    nc.vector.tensor_copy(sgtab[:, c0 * U:(c0 + npr) * U],
                                          ps[:, :npr * U])

            p1_state = dict(done=0)
            P1_AHEAD = 12    # keep sg filled this many blocks ahead of tiles

            def ensure_p1(upto):
                while p1_state['done'] < min(NBLK, upto):
                    emit_p1_pair(p1_state['done'])
                    p1_state['done'] += 2

            # ---- phase 2 ----
            eft_tiles = {}
            oh_tiles = {}
            oht_tiles = {}
            NBLK_eff = 0 if 'phase1' in ablate else NBLK

            def ensure_chunk(g):
                if g in eft_tiles:
                    return
                if 'nodma' in ablate and eft_tiles:
                    g0 = next(iter(eft_tiles))
                    eft_tiles[g] = eft_tiles[g0]
                    oh_tiles[g] = oh_tiles[g0]
                    oht_tiles[g] = oht_tiles[g0]
                    return
                t0 = g * GCH
                n_t = min(GCH, NT - t0)
                et = eftp.tile([FE, GCH * 128], BF16, tag="et", name=f"et{g}")
                nc.sync.dma_start(out=et[:, :n_t * 128],
                                  in_=eft[g, :, :n_t * 128])
                eft_tiles[g] = et
                ot = ohp.tile([128, GCH * 128], FP8, tag="oht", name=f"oh{g}")
                nc.scalar.dma_start(out=ot[:, :n_t * 128],
                                    in_=ohc[g, :, :n_t * 128])
                oh_tiles[g] = ot
                ott = ohtp.tile([128, GCH * 128], FP8, tag="ohtt",
                                name=f"oht{g}")
                nc.gpsimd.dma_start(out=ott[:, :n_t * 128],
                                    in_=ohtc[g, :, :n_t * 128])
                oht_tiles[g] = ott

            LAG = 12
            pending = []          # (oh_ap, msgs_ap, pso_region, start, stop, flush)
            state = dict(pso=None, pso_g0=None)

            def emit_scatter(ent):
                oh_ap, msgs_ap, pso_, st_, sp_, flush = ent
                nc.tensor.matmul(pso_, oh_ap, msgs_ap, start=st_, stop=sp_,
                                 skip_group_check=True)
                if flush is not None:
                    flush()

            # flat tile schedule: (block, type, first/last in block)
            sched = []
            for b in range(NBLK_eff):
                ntile_b = int(K[b].sum())
                done = 0
                for t in range(T):
                    for k in range(int(K[b, t])):
                        sched.append((b, t, done == 0, done == ntile_b - 1))
                        done += 1

            ntiles_of = [int(K[b].sum()) for b in range(NBLK)]
            flush_owner = {}
            for g0 in range(0, NBLK, OB):
                grp = [b for b in range(g0, min(g0 + OB, NBLK))]
                live = [b for b in grp if ntiles_of[b] > 0]
                flush_owner[g0] = live[-1] if live else None

            def group_prolog(g0):
                if state['pso_g0'] == g0:
                    return
                state['pso_g0'] = g0
                nb = min(OB, NBLK - g0)
                state['pso'] = psoutp.tile([128, OB * U], F32, tag="pso",
                                           name=f"pso{g0}")
                for bb in range(g0, g0 + nb):
                    if ntiles_of[bb] == 0:
                        nc.vector.memset(
                            state['pso'][:, (bb - g0) * U:(bb - g0 + 1) * U],
                            0.0)

            def make_flush(g0, pso):
                def flush():
                    nb = min(OB, NBLK - g0)
                    ost = outstp.tile([128, OB * U], BF16, tag="ost",
                                      name=f"ost{g0}")
                    nc.scalar.copy(ost[:, :nb * U], pso[:, :nb * U])
                    nc.sync.dma_start(
                        out=out[g0:g0 + nb, :, :].rearrange("j p u -> p j u"),
                        in_=ost[:, :nb * U].rearrange("p (j u) -> p j u", u=U))
                return flush

            if 'noact' in ablate:
                dummy_msgs = constp.tile([128, U], BF16, tag="dummy")
                nc.vector.memset(dummy_msgs[:, :], 0.25)

            ensure_p1(P1_AHEAD)
            for ii, (b, t, first, last) in enumerate(sched):
                ensure_p1(b + P1_AHEAD)
                g, s = divmod(ii, GCH)
                ensure_chunk(g)
                if ii % GCH == 0:
                    for gg in (g + 1, g + 2, g + 3):
                        if gg * GCH < NT:
                            ensure_chunk(gg)
                ef = eft_tiles[g][:, s * 128:(s + 1) * 128]
                oht_ap = oht_tiles[g][:, s * 128:(s + 1) * 128]
                oh_ap = oh_tiles[g][:, s * 128:(s + 1) * 128]
                gsl = slice(0, U)
                vsl = slice(U, 2 * U)
                bank = bankp.tile([128, 2 * U], F32, tag="bk")
                nc.tensor.matmul(bank[:, :], ef, wcat_sb[t][:, :],
                                 start=True, stop=('nogather' in ablate),
                                 skip_group_check=True)
                if 'nogather' not in ablate:
                    nc.tensor.matmul(bank[:, gsl], oht_ap,
                                     sgtab[:, b * U:(b + 1) * U],
                                     start=False, stop=True,
                                     skip_group_check=True)
                if 'noact' in ablate:
                    # timing diagnostic: decouple PE from the ACT/DVE chain
                    msgs = dummy_msgs
                else:
                    gate = gatep.tile([128, U], BF16)
                    nc.scalar.activation(gate[:, :], bank[:, gsl], AF.Sigmoid)
                    # bias add on DVE (off the PE): val+b -> bf16, then a
                    # cheap bf16x bf16 multiply
                    valb = valbp.tile([128, U], BF16)
                    nc.vector.tensor_tensor(valb[:, :], bank[:, vsl],
                                            btab_sb[:, t * U:(t + 1) * U],
                                            ALU.add)
                    msgs = msgsp.tile([128, U], BF16)
                    nc.vector.tensor_tensor(msgs[:, :], gate[:, :],
                                            valb[:, :], ALU.mult)
                g0 = (b // OB) * OB
                if first:
                    group_prolog(g0)
                if 'noscatter' not in ablate:
                    pso_region = state['pso'][:, (b - g0) * U:(b - g0 + 1) * U]
                    ent = [oh_ap, msgs[:, :], pso_region, first, last, None]
                    if last and flush_owner[g0] == b:
                        ent[5] = make_flush(g0, state['pso'])
                    pending.append(ent)
                    if len(pending) > LAG:
                        emit_scatter(pending.pop(0))
            ensure_p1(NBLK)     # sg for any trailing edge-less blocks
            for ent in pending:
                emit_scatter(ent)
            # groups consisting entirely of empty blocks
            if NBLK_eff:
                for g0 in range(0, NBLK, OB):
                    if flush_owner[g0] is None:
                        group_prolog(g0)
                        make_flush(g0, state['pso'])()
            if bench_iters:
                loop_cm.__exit__(None, None, None)
    nc.compile()
    return nc


def make_const_inputs(cfg: Cfg, W_gate, W_gate_e, W_dense, b_dense):
    FKC = cfg.F // 128
    T, U = cfg.T, cfg.U
    btab_np = np.broadcast_to(
        np.asarray(b_dense, np.float32).reshape(1, T * U), (128, T * U))
    return dict(
        wg=_bf(np.ascontiguousarray(
            np.asarray(W_gate, np.float32).reshape(FKC, 128, cfg.U))),
        wcat=_bf(np.concatenate([np.asarray(W_gate_e, np.float32),
                                 np.asarray(W_dense, np.float32)], axis=2)),
        btab=_bf(btab_np.copy()),
    )


def make_in_maps(cfg: Cfg, inputs):
    K, NT, per_core, slot_of_node = preprocess(
        cfg, inputs['edge_idx'], inputs['edge_feats'])
    feat_in = make_feat_inputs(cfg, inputs['features'], slot_of_node)
    const_in = make_const_inputs(cfg, inputs['W_gate'], inputs['W_gate_e'],
                                 inputs['W_dense'], inputs['b_dense'])
    in_maps = []
    for c in range(cfg.ncores):
        m = dict(const_in)
        m['featT'] = feat_in[c]
        m.update(per_core[c])
        in_maps.append(m)
    return K, NT, in_maps, slot_of_node


def extract_output(cfg: Cfg, results, slot_of_node):
    out_full = np.zeros((cfg.ncores * cfg.R, cfg.U), np.float32)
    for c in range(cfg.ncores):
        dev = np.asarray(results[c]['out']).astype(np.float32)
        dev = dev.reshape(cfg.RS, cfg.U)
        out_full[c * cfg.R:(c + 1) * cfg.R] = dev[slot_of_node[c]]
    return out_full[:cfg.BN]


def run_full(cfg: Cfg, inputs, run_fn):
    K, NT, in_maps, slot_of_node = make_in_maps(cfg, inputs)
    nc = build_kernel(cfg, K, NT)
    results = run_fn(nc, in_maps)
    return extract_output(cfg, results, slot_of_node)


# ============================================================================
# Self-contained entry point (harness contract):
#   kernel(**inputs) takes the FULL unsharded inputs and returns the FULL
#   output [2, 50000, 256] float32. Internally: node-shard across the 8
#   NeuronCores (no collectives needed since gather idx == scatter idx per
#   edge), compile one SPMD Bass program, run via run_bass_kernel_spmd.
# ============================================================================
from concourse.bass_utils import run_bass_kernel_spmd

_CACHE = {}


def kernel(features, edge_idx, edge_feats, W_gate, W_gate_e, W_dense, b_dense):
    features = np.asarray(features)
    edge_idx = np.asarray(edge_idx)
    edge_feats = np.asarray(edge_feats)
    B, N, F = features.shape
    BN = B * N
    cfg = Cfg(ncores=8, R=-(-BN // (8 * 128)) * 128, F=F,
              U=np.asarray(W_gate).shape[1], FE=edge_feats.shape[2],
              T=edge_feats.shape[0], BN=BN)
    cfg.NB = -(-cfg.R // 128) + 34      # packing slack (~35% spare slots)

    inputs = dict(features=features, edge_idx=edge_idx, edge_feats=edge_feats,
                  W_gate=W_gate, W_gate_e=W_gate_e, W_dense=W_dense,
                  b_dense=b_dense)
    K, NT, in_maps, slot_of_node = make_in_maps(cfg, inputs)

    key = (cfg.R, cfg.NB, cfg.F, cfg.U, cfg.FE, cfg.T, NT, K.tobytes())
    nc = _CACHE.get(key)
    if nc is None:
        nc = build_kernel(cfg, K, NT)
        _CACHE[key] = nc

    res = run_bass_kernel_spmd(nc, in_maps, core_ids=list(range(cfg.ncores)))
    out = extract_output(cfg, res.results, slot_of_node)
    return out.reshape(B, N, cfg.U).astype(np.float32)


# revision 41
# speedup vs baseline: 1.1115x; 1.1115x over previous
"""Sharded Bass kernel for nn_AggrGATGated: gated GNN message passing.

Sharding: nodes are sharded across the 8 cores. Each edge's gather index ==
its scatter index (the reference gathers src_gated[edge_idx] and scatter-adds
to the same edge_idx), so a core that owns a node range processes exactly the
edges targeting it: NO collectives are needed at all.

Within a core, nodes are re-packed into NB blocks of 128 PSUM slots by a
worst-fit-decreasing bin packer so that each (block, edge-type) holds <= 128
edges; this makes every (block, type) exactly one 128-edge matmul tile.
The host precomputes, per core: transposed 128x128 edge-feature tiles
(chunk-packed for large DMAs) and BOTH orientations of each tile's one-hot
edge->slot matrix in fp8 (exact for 0/1 values; fp8 stationary x bf16 moving
matmuls are legal on TRN2).

Real-hardware cost structure (measured on-device, which diverges from the
TimelineSim cost model): contiguous-rhs bf16 matmuls stream ~2 cols/cycle,
strided rhs APs halve the rate, K=1 matmuls force a PE tile_size reconfig,
and per-queue DMA caps at ~140-165 GB/s. The kernel therefore uses three
contiguous matmuls per tile via a per-tile [gate|val] PSUM bank:
  mm1: bank[e, 0:2U]  = efT.T @ [W_gate_e[t] | W_dense[t]]   (N=512, start)
  mm2: bank[e, 0:U]  += ohT.T @ sg_block_b                   (N=256, stop)
  gate = sigmoid(bank[:, 0:U]) -> bf16                  (ACT)
  valb = bank[:, U:2U] + btab_t -> bf16                 (DVE; bias off-PE)
  msgs = gate * valb                                    (DVE, bf16 2x mode)
  mm3: pso[b%2] += oh.T @ msgs    (N=256, PSUM-accumulated, LAG-deferred)
Per 2-block group: one f32->bf16 flush copy (ACT) + one grouped DMA store;
output is bf16, upcast to f32 on host.

Phase 1 fills the SBUF-resident sg table (sg = features_shard @ W_gate, two
blocks per PSUM bank, copies alternating ACT/DVE) and is emitted interleaved
into the phase-2 tile stream (P1_AHEAD blocks ahead) so its featT DMA and
matmuls overlap phase-2 compute instead of serializing in the in-order PE
queue. DMA queues: eft + out stores on SP, featT + oh on ACT(HWDGE), ohT on
the GPSIMD SWDGE ring as a third channel.
"""
import dataclasses
import numpy as np
import ml_dtypes

def _bf(x):
    return np.asarray(x).astype(ml_dtypes.bfloat16)

def _f8(x):
    return np.asarray(x).astype(ml_dtypes.float8_e4m3)

import concourse.bass as bass
import concourse.bacc as bacc
import concourse.mybir as mybir
from concourse.tile import TileContext

F32 = mybir.dt.float32
I32 = mybir.dt.int32
BF16 = mybir.dt.bfloat16
FP8 = mybir.dt.float8e4
AF = mybir.ActivationFunctionType
ALU = mybir.AluOpType


@dataclasses.dataclass
class Cfg:
    ncores: int = 8
    R: int = 12544          # real node coverage per core (ceil(BN/8) to 128)
    NB: int = 132           # device blocks per core (>= R/128; slack for packing)
    F: int = 256            # node feature dim
    U: int = 256            # output dim
    FE: int = 128           # edge feature dim
    T: int = 3              # edge types
    BN: int = 100000        # real node count (B*N)
    GCH: int = 16           # eft/oh chunk, in tiles
    JB: int = 8             # featT/sg blocks per DMA group
    OB: int = 2             # out blocks per psum group / DMA store

    @property
    def NBLK(self):
        return self.NB

    @property
    def RS(self):
        return self.NB * 128    # device node slots per core


def _pack_core(d: np.ndarray, NB: int, cap: int = 128):
    """Assign nodes (degree vectors d [Rn, T]) to NB blocks of <=128 slots with
    per-type edge-count <= cap. Worst-fit decreasing; overflows allowed (they
    just bump the tile count). Returns assign [Rn]."""
    Rn, T = d.shape
    order = np.argsort(-d.sum(axis=1), kind='stable')
    rem = np.full((NB, T), cap, np.int64)
    slots = np.full(NB, 128, np.int64)
    assign = np.empty(Rn, np.int64)
    for n in order:
        dn = d[n]
        fits = (rem >= dn).all(axis=1) & (slots > 0)
        if fits.any():
            score = (rem - dn).min(axis=1).astype(np.float64)
            score[~fits] = -np.inf
            b = int(np.argmax(score * 128 + slots))
        else:
            ok = slots > 0
            over = np.maximum(dn - rem, 0).sum(axis=1).astype(np.float64)
            over[~ok] = np.inf
            b = int(np.argmin(over))
        assign[n] = b
        rem[b] -= dn
        slots[b] -= 1
    return assign


def preprocess(cfg: Cfg, edge_idx: np.ndarray, edge_feats: np.ndarray):
    """Pack nodes into blocks, bucket edges per (block, type) tile.

    Returns (K, NT, per_core, slot_of_node) where per_core holds the device
    input arrays and slot_of_node [NC, R] maps local node -> device slot."""
    NC, R, NB, T, FE, GCH = cfg.ncores, cfg.R, cfg.NB, cfg.T, cfg.FE, cfg.GCH
    edge_idx = np.asarray(edge_idx)

    # per-node type degrees over the padded node space
    deg = np.zeros((NC * R, T), np.int32)
    for t in range(T):
        deg[:, t] = np.bincount(edge_idx[t], minlength=NC * R)[:NC * R]

    slot_of_node = np.zeros((NC, R), np.int64)
    for c in range(NC):
        assign = _pack_core(deg[c * R:(c + 1) * R], NB)
        order = np.argsort(assign, kind='stable')
        ranks = np.empty(R, np.int64)
        # rank within block
        blocksorted = assign[order]
        start = np.searchsorted(blocksorted, np.arange(NB))
        pos = np.arange(R) - start[blocksorted]
        ranks[order] = pos
        slot_of_node[c] = assign * 128 + ranks

    # per (core, block, type) counts using slots
    counts = np.zeros((NC, NB, T), np.int64)
    eslots = []          # per t: (sorted edge ids, their cores, their slots)
    for t in range(T):
        idx = edge_idx[t]
        core = idx // R
        loc = idx - core * R
        slot = slot_of_node[core, loc]
        key = core * (NB * 128) + slot
        o = np.argsort(key, kind='stable')
        eslots.append((o, core[o], slot[o]))
        blk = core[o] * NB + (slot[o] >> 7)
        cnt = np.bincount(blk, minlength=NC * NB)
        counts[:, :, t] = cnt.reshape(NC, NB)

    K = -(-counts.max(axis=0) // 128)        # [NB, T], may contain 0
    NT = int(K.sum())
    NCH = -(-NT // GCH)
    Kcum = np.zeros((NB, T), np.int64)
    acc = 0
    for b in range(NB):
        for t in range(T):
            Kcum[b, t] = acc
            acc += int(K[b, t])

    per_core = []
    for c in range(NC):
        ids = np.full((NT, 128), -1, dtype=np.int64)
        offs = np.full((NT, 128), -1, dtype=np.int64)
        for t in range(T):
            o, ecore, eslot = eslots[t]
            lo = np.searchsorted(ecore, c)
            hi = np.searchsorted(ecore, c + 1)
            sl = eslot[lo:hi]
            eid = o[lo:hi]
            bounds = np.searchsorted(sl, np.arange(NB + 1) * 128)
            for b in range(NB):
                s, e = bounds[b], bounds[b + 1]
                n = e - s
                if n == 0:
                    continue
                ti = int(Kcum[b, t])
                for k in range(int(K[b, t])):
                    a0, a1 = k * 128, min((k + 1) * 128, n)
                    m = a1 - a0
                    if m <= 0:
                        break
                    ids[ti + k, :m] = eid[s + a0:s + a1]
                    offs[ti + k, :m] = sl[s + a0:s + a1] & 127
        # eft tiles (transposed), chunk-major packing
        type_of_tile = np.zeros(NT, np.int64)
        for b in range(NB):
            for t in range(T):
                ti = int(Kcum[b, t])
                type_of_tile[ti:ti + int(K[b, t])] = t
        eft = np.zeros((NT, 128, FE), dtype=np.float32)
        for t in range(T):
            sel = np.nonzero(type_of_tile == t)[0]
            idsf = ids[sel]
            v = idsf >= 0
            ef = np.zeros((len(sel), 128, FE), np.float32)
            ef[v] = np.asarray(edge_feats[t])[idsf[v]]
            eft[sel] = ef
        eftT = eft.transpose(0, 2, 1)
        eftC = np.zeros((NCH, FE, GCH * 128), ml_dtypes.bfloat16)
        for ch in range(NCH):
            n_t = min(GCH, NT - ch * GCH)
            blk = eftT[ch * GCH: ch * GCH + n_t]
            eftC[ch, :, :n_t * 128] = blk.transpose(1, 0, 2).reshape(FE, n_t * 128)
        # one-hot (edge->slot) tiles in fp8, both orientations, chunk-packed
        oh = np.zeros((NT, 128, 128), np.float32)   # [tile, edge, slot]
        tt, ee = np.nonzero(offs >= 0)
        oh[tt, ee, offs[tt, ee]] = 1.0
        ohT = oh.transpose(0, 2, 1)                 # [tile, slot, edge]
        ohC = np.zeros((NCH, 128, GCH * 128), ml_dtypes.float8_e4m3)
        ohTC = np.zeros((NCH, 128, GCH * 128), ml_dtypes.float8_e4m3)
        for ch in range(NCH):
            n_t = min(GCH, NT - ch * GCH)
            blk = oh[ch * GCH: ch * GCH + n_t]
            ohC[ch, :, :n_t * 128] = _f8(
                blk.transpose(1, 0, 2).reshape(128, n_t * 128))
            blkT = ohT[ch * GCH: ch * GCH + n_t]
            ohTC[ch, :, :n_t * 128] = _f8(
                blkT.transpose(1, 0, 2).reshape(128, n_t * 128))
        per_core.append(dict(eft=eftC, ohc=ohC, ohtc=ohTC))
    return K, NT, per_core, slot_of_node


def make_feat_inputs(cfg: Cfg, features: np.ndarray, slot_of_node: np.ndarray):
    """Per-core packed featT over device slots: [NBJ, 128, JB*FKC*128]."""
    NC, R, F, JB, NB = cfg.ncores, cfg.R, cfg.F, cfg.JB, cfg.NB
    FKC = F // 128
    RS = cfg.RS
    NBJ = -(-NB // JB)
    feat_flat = np.asarray(features).reshape(-1, F)
    outs = []
    for c in range(NC):
        fs = np.zeros((RS, F), np.float32)
        lo, hi = c * R, min((c + 1) * R, feat_flat.shape[0])
        if hi > lo:
            fs[slot_of_node[c][:hi - lo]] = feat_flat[lo:hi]
        fc = fs.reshape(NB, 128, FKC, 128)
        ft = fc.transpose(0, 2, 3, 1)                # [NB, FKC, f, n]
        packed = np.zeros((NBJ, 128, JB * FKC * 128), ml_dtypes.bfloat16)
        for jc in range(NBJ):
            nb = min(JB, NB - jc * JB)
            blk = ft[jc * JB: jc * JB + nb]
            packed[jc, :, :nb * FKC * 128] = (
                blk.transpose(2, 0, 1, 3).reshape(128, nb * FKC * 128))
        outs.append(packed)
    return outs


def build_kernel(cfg: Cfg, K: np.ndarray, NT: int, dbg: bool = False, bench_iters: int = 0, ablate: str = ''):
    NBLK, T, U, FE, F = cfg.NBLK, cfg.T, cfg.U, cfg.FE, cfg.F
    GCH, JB, OB = cfg.GCH, cfg.JB, cfg.OB
    FKC = F // 128
    NCH = -(-NT // GCH)
    NBJ = -(-NBLK // JB)

    nc = bacc.Bacc("TRN2", target_bir_lowering=False, debug=False,
                   num_devices=cfg.ncores)

    featT = nc.dram_tensor("featT", [NBJ, 128, JB * FKC * 128], BF16,
                           kind="ExternalInput")
    wg = nc.dram_tensor("wg", [FKC, 128, U], BF16, kind="ExternalInput")
    wcat = nc.dram_tensor("wcat", [T, FE, 2 * U], BF16, kind="ExternalInput")
    btab = nc.dram_tensor("btab", [128, T * U], BF16, kind="ExternalInput")
    eft = nc.dram_tensor("eft", [NCH, FE, GCH * 128], BF16, kind="ExternalInput")
    ohc = nc.dram_tensor("ohc", [NCH, 128, GCH * 128], FP8, kind="ExternalInput")
    ohtc = nc.dram_tensor("ohtc", [NCH, 128, GCH * 128], FP8,
                          kind="ExternalInput")
    out = nc.dram_tensor("out", [NBLK, 128, U], BF16, kind="ExternalOutput")

    with TileContext(nc) as tc:
        with (
            tc.tile_pool(name="const", bufs=1) as constp,
            tc.tile_pool(name="ftile", bufs=3) as ftp,
            tc.tile_pool(name="eftl", bufs=6) as eftp,
            tc.tile_pool(name="ohl", bufs=6) as ohp,
            tc.tile_pool(name="ohtl", bufs=6) as ohtp,
            tc.tile_pool(name="gate", bufs=10) as gatep,
            tc.tile_pool(name="valb", bufs=10) as valbp,
            tc.tile_pool(name="msgs", bufs=(22 if "lag18" in ablate else 12)) as msgsp,
            tc.tile_pool(name="outst", bufs=2) as outstp,
            tc.tile_pool(name="bank", bufs=6, space="PSUM") as bankp,
            tc.tile_pool(name="psout", bufs=2, space="PSUM") as psoutp,
        ):
            # ---- constants ----
            wg_sb = []
            for kc in range(FKC):
                w = constp.tile([128, U], BF16, tag=f"wg{kc}")
                nc.scalar.dma_start(out=w[:, :], in_=wg[kc, :, :])
                wg_sb.append(w)
            wcat_sb = []
            for t in range(T):
                w = constp.tile([FE, 2 * U], BF16, tag=f"wcat{t}")
                nc.scalar.dma_start(out=w[:, :], in_=wcat[t, :, :])
                wcat_sb.append(w)
            btab_sb = constp.tile([128, T * U], BF16, tag="btab")
            nc.scalar.dma_start(out=btab_sb[:, :], in_=btab[:, :])

            sgtab = constp.tile([128, NBLK * U], BF16, tag="sgtab")

            # ---- phase 1 emitters (interleaved into the phase-2 stream) ----
            import contextlib
            loop_cm = (tc.For_i(0, bench_iters, 1, hint_engines=(
                mybir.EngineType.PE, mybir.EngineType.DVE,
                mybir.EngineType.Activation, mybir.EngineType.Pool,
                mybir.EngineType.SP))
                if bench_iters else contextlib.nullcontext())
            loop_ctx = loop_cm.__enter__() if bench_iters else None

            ft_tiles = {}

            def ensure_ft(jc):
                if jc in ft_tiles or jc >= NBJ:
                    return
                nb = min(JB, NBLK - jc * JB)
                ft = ftp.tile([128, JB * FKC * 128], BF16, tag="ft",
                              name=f"ft{jc}")
                nc.scalar.dma_start(out=ft[:, :nb * FKC * 128],
                                    in_=featT[jc, :, :nb * FKC * 128])
                ft_tiles[jc] = ft

            def emit_p1_pair(j0):
                """sg for blocks j0, j0+1 (paired in one PSUM bank)."""
                jc = j0 // JB
                ensure_ft(jc)
                ensure_ft(jc + 1)
                ft = ft_tiles[jc]
                jj = j0 - jc * JB
                npr = min(2, NBLK - j0)
                ps = bankp.tile([128, 2 * U], F32, tag="bk", name=f"p1_{j0}")
                for d in range(npr):
                    for kc in range(FKC):
                        o = ((jj + d) * FKC + kc) * 128
                        nc.tensor.matmul(ps[:, d * U:(d + 1) * U],
                                         ft[:, o:o + 128], wg_sb[kc][:, :],
                                         start=(kc == 0),
                                         stop=(kc == FKC - 1),
                                         skip_group_check=True)
                c0 = j0
                if (j0 // 2) % 2 == 0:
                    nc.scalar.copy(sgtab[:, c0 * U:(c0 + npr) * U],
                                   ps[:, :npr * U])
                else:
                    nc.vector.tensor_copy(sgtab[:, c0 * U:(c0 + npr) * U],
                                          ps[:, :npr * U])

            p1_state = dict(done=0)
            P1_AHEAD = 12    # keep sg filled this many blocks ahead of tiles

            def ensure_p1(upto):
                while p1_state['done'] < min(NBLK, upto):
                    emit_p1_pair(p1_state['done'])
                    p1_state['done'] += 2

            # ---- phase 2 ----
            eft_tiles = {}
            oh_tiles = {}
            oht_tiles = {}
            NBLK_eff = 0 if 'phase1' in ablate else NBLK

            def ensure_chunk(g):
                if g in eft_tiles:
                    return
                if 'nodma' in ablate and eft_tiles:
                    g0 = next(iter(eft_tiles))
                    eft_tiles[g] = eft_tiles[g0]
                    oh_tiles[g] = oh_tiles[g0]
                    oht_tiles[g] = oht_tiles[g0]
                    return
                t0 = g * GCH
                n_t = min(GCH, NT - t0)
                et = eftp.tile([FE, GCH * 128], BF16, tag="et", name=f"et{g}")
                nc.sync.dma_start(out=et[:, :n_t * 128],
                                  in_=eft[g, :, :n_t * 128])
                eft_tiles[g] = et
                ot = ohp.tile([128, GCH * 128], FP8, tag="oht", name=f"oh{g}")
                nc.scalar.dma_start(out=ot[:, :n_t * 128],
                                    in_=ohc[g, :, :n_t * 128])
                oh_tiles[g] = ot
                ott = ohtp.tile([128, GCH * 128], FP8, tag="ohtt",
                                name=f"oht{g}")
                nc.gpsimd.dma_start(out=ott[:, :n_t * 128],
                                    in_=ohtc[g, :, :n_t * 128])
                oht_tiles[g] = ott

            LAG = 18 if "lag18" in ablate else 12
            pending = []          # (oh_ap, msgs_ap, pso_region, start, stop, flush)
            state = dict(pso=None, pso_g0=None)

            def emit_scatter(ent):
                oh_ap, msgs_ap, pso_, st_, sp_, flush = ent
                nc.tensor.matmul(pso_, oh_ap, msgs_ap, start=st_, stop=sp_,
                                 skip_group_check=True)
                if flush is not None:
                    flush()

            # flat tile schedule: (block, type, first/last in block)
            sched = []
            for b in range(NBLK_eff):
                ntile_b = int(K[b].sum())
                done = 0
                for t in range(T):
                    for k in range(int(K[b, t])):
                        sched.append((b, t, done == 0, done == ntile_b - 1))
                        done += 1

            ntiles_of = [int(K[b].sum()) for b in range(NBLK)]
            flush_owner = {}
            for g0 in range(0, NBLK, OB):
                grp = [b for b in range(g0, min(g0 + OB, NBLK))]
                live = [b for b in grp if ntiles_of[b] > 0]
                flush_owner[g0] = live[-1] if live else None

            def group_prolog(g0):
                if state['pso_g0'] == g0:
                    return
                state['pso_g0'] = g0
                nb = min(OB, NBLK - g0)
                state['pso'] = psoutp.tile([128, OB * U], F32, tag="pso",
                                           name=f"pso{g0}")
                for bb in range(g0, g0 + nb):
                    if ntiles_of[bb] == 0:
                        nc.vector.memset(
                            state['pso'][:, (bb - g0) * U:(bb - g0 + 1) * U],
                            0.0)

            def make_flush(g0, pso):
                def flush():
                    nb = min(OB, NBLK - g0)
                    ost = outstp.tile([128, OB * U], BF16, tag="ost",
                                      name=f"ost{g0}")
                    nc.scalar.copy(ost[:, :nb * U], pso[:, :nb * U])
                    oq = nc.gpsimd if 'outswdge' in ablate else nc.sync
                    oq.dma_start(
                        out=out[g0:g0 + nb, :, :].rearrange("j p u -> p j u"),
                        in_=ost[:, :nb * U].rearrange("p (j u) -> p j u", u=U))
                return flush

            if 'noact' in ablate:
                dummy_msgs = constp.tile([128, U], BF16, tag="dummy")
                nc.vector.memset(dummy_msgs[:, :], 0.25)

            ensure_p1(P1_AHEAD)
            for ii, (b, t, first, last) in enumerate(sched):
                ensure_p1(b + P1_AHEAD)
                g, s = divmod(ii, GCH)
                ensure_chunk(g)
                if ii % GCH == 0:
                    for gg in (g + 1, g + 2, g + 3):
                        if gg * GCH < NT:
                            ensure_chunk(gg)
                ef = eft_tiles[g][:, s * 128:(s + 1) * 128]
                oht_ap = oht_tiles[g][:, s * 128:(s + 1) * 128]
                oh_ap = oh_tiles[g][:, s * 128:(s + 1) * 128]
                gsl = slice(0, U)
                vsl = slice(U, 2 * U)
                bank = bankp.tile([128, 2 * U], F32, tag="bk")
                nc.tensor.matmul(bank[:, :], ef, wcat_sb[t][:, :],
                                 start=True, stop=('nogather' in ablate),
                                 skip_group_check=True)
                if 'nogather' not in ablate:
                    nc.tensor.matmul(bank[:, gsl], oht_ap,
                                     sgtab[:, b * U:(b + 1) * U],
                                     start=False, stop=True,
                                     skip_group_check=True)
                if 'noact' in ablate:
                    # timing diagnostic: decouple PE from the ACT/DVE chain
                    msgs = dummy_msgs
                else:
                    gate = gatep.tile([128, U], BF16)
                    nc.scalar.activation(gate[:, :], bank[:, gsl], AF.Sigmoid)
                    # bias add on DVE (off the PE): val+b -> bf16, then a
                    # cheap bf16x bf16 multiply
                    valb = valbp.tile([128, U], BF16)
                    nc.vector.tensor_tensor(valb[:, :], bank[:, vsl],
                                            btab_sb[:, t * U:(t + 1) * U],
                                            ALU.add)
                    msgs = msgsp.tile([128, U], BF16)
                    nc.vector.tensor_tensor(msgs[:, :], gate[:, :],
                                            valb[:, :], ALU.mult)
                g0 = (b // OB) * OB
                if first:
                    group_prolog(g0)
                if 'noscatter' not in ablate:
                    pso_region = state['pso'][:, (b - g0) * U:(b - g0 + 1) * U]
                    ent = [oh_ap, msgs[:, :], pso_region, first, last, None]
                    if last and flush_owner[g0] == b:
                        ent[5] = make_flush(g0, state['pso'])
                    pending.append(ent)
                    if len(pending) > LAG:
                        emit_scatter(pending.pop(0))
            ensure_p1(NBLK)     # sg for any trailing edge-less blocks
            for ent in pending:
                emit_scatter(ent)
            # groups consisting entirely of empty blocks
            if NBLK_eff:
                for g0 in range(0, NBLK, OB):
                    if flush_owner[g0] is None:
                        group_prolog(g0)
                        make_flush(g0, state['pso'])()
            if bench_iters:
                loop_cm.__exit__(None, None, None)
    nc.compile()
    return nc


def make_const_inputs(cfg: Cfg, W_gate, W_gate_e, W_dense, b_dense):
    FKC = cfg.F // 128
    T, U = cfg.T, cfg.U
    btab_np = np.broadcast_to(
        np.asarray(b_dense, np.float32).reshape(1, T * U), (128, T * U))
    return dict(
        wg=_bf(np.ascontiguousarray(
            np.asarray(W_gate, np.float32).reshape(FKC, 128, cfg.U))),
        wcat=_bf(np.concatenate([np.asarray(W_gate_e, np.float32),
                                 np.asarray(W_dense, np.float32)], axis=2)),
        btab=_bf(btab_np.copy()),
    )


def make_in_maps(cfg: Cfg, inputs):
    K, NT, per_core, slot_of_node = preprocess(
        cfg, inputs['edge_idx'], inputs['edge_feats'])
    feat_in = make_feat_inputs(cfg, inputs['features'], slot_of_node)
    const_in = make_const_inputs(cfg, inputs['W_gate'], inputs['W_gate_e'],
                                 inputs['W_dense'], inputs['b_dense'])
    in_maps = []
    for c in range(cfg.ncores):
        m = dict(const_in)
        m['featT'] = feat_in[c]
        m.update(per_core[c])
        in_maps.append(m)
    return K, NT, in_maps, slot_of_node


def extract_output(cfg: Cfg, results, slot_of_node):
    out_full = np.zeros((cfg.ncores * cfg.R, cfg.U), np.float32)
    for c in range(cfg.ncores):
        dev = np.asarray(results[c]['out']).astype(np.float32)
        dev = dev.reshape(cfg.RS, cfg.U)
        out_full[c * cfg.R:(c + 1) * cfg.R] = dev[slot_of_node[c]]
    return out_full[:cfg.BN]


def run_full(cfg: Cfg, inputs, run_fn):
    K, NT, in_maps, slot_of_node = make_in_maps(cfg, inputs)
    nc = build_kernel(cfg, K, NT)
    results = run_fn(nc, in_maps)
    return extract_output(cfg, results, slot_of_node)


# ============================================================================
# Self-contained entry point (harness contract):
#   kernel(**inputs) takes the FULL unsharded inputs and returns the FULL
#   output [2, 50000, 256] float32. Internally: node-shard across the 8
#   NeuronCores (no collectives needed since gather idx == scatter idx per
#   edge), compile one SPMD Bass program, run via run_bass_kernel_spmd.
# ============================================================================
from concourse.bass_utils import run_bass_kernel_spmd

_CACHE = {}


def kernel(features, edge_idx, edge_feats, W_gate, W_gate_e, W_dense, b_dense):
    features = np.asarray(features)
    edge_idx = np.asarray(edge_idx)
    edge_feats = np.asarray(edge_feats)
    B, N, F = features.shape
    BN = B * N
    cfg = Cfg(ncores=8, R=-(-BN // (8 * 128)) * 128, F=F,
              U=np.asarray(W_gate).shape[1], FE=edge_feats.shape[2],
              T=edge_feats.shape[0], BN=BN)
    cfg.NB = -(-cfg.R // 128) + 34      # packing slack (~35% spare slots)

    inputs = dict(features=features, edge_idx=edge_idx, edge_feats=edge_feats,
                  W_gate=W_gate, W_gate_e=W_gate_e, W_dense=W_dense,
                  b_dense=b_dense)
    K, NT, in_maps, slot_of_node = make_in_maps(cfg, inputs)

    key = (cfg.R, cfg.NB, cfg.F, cfg.U, cfg.FE, cfg.T, NT, K.tobytes())
    nc = _CACHE.get(key)
    if nc is None:
        nc = build_kernel(cfg, K, NT)
        _CACHE[key] = nc

    res = run_bass_kernel_spmd(nc, in_maps, core_ids=list(range(cfg.ncores)))
    out = extract_output(cfg, res.results, slot_of_node)
    return out.reshape(B, N, cfg.U).astype(np.float32)


# revision 42
# speedup vs baseline: 1.1869x; 1.0678x over previous
"""Sharded Bass kernel for nn_AggrGATGated: gated GNN message passing.

Sharding: nodes are sharded across the 8 cores. Each edge's gather index ==
its scatter index (the reference gathers src_gated[edge_idx] and scatter-adds
to the same edge_idx), so a core that owns a node range processes exactly the
edges targeting it: NO collectives are needed at all.

Within a core, nodes are re-packed into NB blocks of 128 PSUM slots by a
worst-fit-decreasing bin packer so that each (block, edge-type) holds <= 128
edges; this makes every (block, type) exactly one 128-edge matmul tile.
The host precomputes, per core: transposed 128x128 edge-feature tiles
(chunk-packed for large DMAs) and BOTH orientations of each tile's one-hot
edge->slot matrix in fp8 (exact for 0/1 values; fp8 stationary x bf16 moving
matmuls are legal on TRN2).

Real-hardware cost structure (measured on-device, which diverges from the
TimelineSim cost model): contiguous-rhs bf16 matmuls stream ~2 cols/cycle,
strided rhs APs halve the rate, K=1 matmuls force a PE tile_size reconfig,
and per-queue DMA caps at ~140-165 GB/s. The kernel therefore uses three
contiguous matmuls per tile via a per-tile [gate|val] PSUM bank:
  mm1: bank[e, 0:2U]  = efT.T @ [W_gate_e[t] | W_dense[t]]   (N=512, start)
  mm2: bank[e, 0:U]  += ohT.T @ sg_block_b                   (N=256, stop)
  gate = sigmoid(bank[:, 0:U]) -> bf16                  (ACT)
  valb = bank[:, U:2U] + btab_t -> bf16                 (DVE; bias off-PE)
  msgs = gate * valb                                    (DVE, bf16 2x mode)
  mm3: pso[b%2] += oh.T @ msgs    (N=256, PSUM-accumulated, LAG-deferred)
Per 2-block group: one f32->bf16 flush copy (ACT) + one grouped DMA store;
output is bf16, upcast to f32 on host.

Phase 1 fills the SBUF-resident sg table (sg = features_shard @ W_gate, two
blocks per PSUM bank, copies alternating ACT/DVE) and is emitted interleaved
into the phase-2 tile stream (P1_AHEAD blocks ahead) so its featT DMA and
matmuls overlap phase-2 compute instead of serializing in the in-order PE
queue. DMA queues: eft + out stores on SP, featT + oh on ACT(HWDGE), ohT on
the GPSIMD SWDGE ring as a third channel.
"""
import dataclasses
import numpy as np
import ml_dtypes

def _bf(x):
    return np.asarray(x).astype(ml_dtypes.bfloat16)

def _f8(x):
    return np.asarray(x).astype(ml_dtypes.float8_e4m3)

import concourse.bass as bass
import concourse.bacc as bacc
import concourse.mybir as mybir
from concourse.tile import TileContext

F32 = mybir.dt.float32
I32 = mybir.dt.int32
BF16 = mybir.dt.bfloat16
FP8 = mybir.dt.float8e4
AF = mybir.ActivationFunctionType
ALU = mybir.AluOpType


@dataclasses.dataclass
class Cfg:
    ncores: int = 8
    R: int = 12544          # real node coverage per core (ceil(BN/8) to 128)
    NB: int = 132           # device blocks per core (>= R/128; slack for packing)
    F: int = 256            # node feature dim
    U: int = 256            # output dim
    FE: int = 128           # edge feature dim
    T: int = 3              # edge types
    BN: int = 100000        # real node count (B*N)
    GCH: int = 16           # eft/oh chunk, in tiles
    JB: int = 8             # featT/sg blocks per DMA group
    OB: int = 2             # out blocks per psum group / DMA store

    @property
    def NBLK(self):
        return self.NB

    @property
    def RS(self):
        return self.NB * 128    # device node slots per core


def _pack_core(d: np.ndarray, NB: int, cap: int = 128):
    """Assign nodes (degree vectors d [Rn, T]) to NB blocks of <=128 slots with
    per-type edge-count <= cap. Worst-fit decreasing; overflows allowed (they
    just bump the tile count). Returns assign [Rn]."""
    Rn, T = d.shape
    order = np.argsort(-d.sum(axis=1), kind='stable')
    rem = np.full((NB, T), cap, np.int64)
    slots = np.full(NB, 128, np.int64)
    assign = np.empty(Rn, np.int64)
    for n in order:
        dn = d[n]
        fits = (rem >= dn).all(axis=1) & (slots > 0)
        if fits.any():
            score = (rem - dn).min(axis=1).astype(np.float64)
            score[~fits] = -np.inf
            b = int(np.argmax(score * 128 + slots))
        else:
            ok = slots > 0
            over = np.maximum(dn - rem, 0).sum(axis=1).astype(np.float64)
            over[~ok] = np.inf
            b = int(np.argmin(over))
        assign[n] = b
        rem[b] -= dn
        slots[b] -= 1
    return assign


def preprocess(cfg: Cfg, edge_idx: np.ndarray, edge_feats: np.ndarray):
    """Pack nodes into blocks, bucket edges per (block, type) tile.

    Returns (K, NT, per_core, slot_of_node) where per_core holds the device
    input arrays and slot_of_node [NC, R] maps local node -> device slot."""
    NC, R, NB, T, FE, GCH = cfg.ncores, cfg.R, cfg.NB, cfg.T, cfg.FE, cfg.GCH
    edge_idx = np.asarray(edge_idx)

    # per-node type degrees over the padded node space
    deg = np.zeros((NC * R, T), np.int32)
    for t in range(T):
        deg[:, t] = np.bincount(edge_idx[t], minlength=NC * R)[:NC * R]

    slot_of_node = np.zeros((NC, R), np.int64)
    for c in range(NC):
        assign = _pack_core(deg[c * R:(c + 1) * R], NB)
        order = np.argsort(assign, kind='stable')
        ranks = np.empty(R, np.int64)
        # rank within block
        blocksorted = assign[order]
        start = np.searchsorted(blocksorted, np.arange(NB))
        pos = np.arange(R) - start[blocksorted]
        ranks[order] = pos
        slot_of_node[c] = assign * 128 + ranks

    # per (core, block, type) counts using slots
    counts = np.zeros((NC, NB, T), np.int64)
    eslots = []          # per t: (sorted edge ids, their cores, their slots)
    for t in range(T):
        idx = edge_idx[t]
        core = idx // R
        loc = idx - core * R
        slot = slot_of_node[core, loc]
        key = core * (NB * 128) + slot
        o = np.argsort(key, kind='stable')
        eslots.append((o, core[o], slot[o]))
        blk = core[o] * NB + (slot[o] >> 7)
        cnt = np.bincount(blk, minlength=NC * NB)
        counts[:, :, t] = cnt.reshape(NC, NB)

    K = -(-counts.max(axis=0) // 128)        # [NB, T], may contain 0
    NT = int(K.sum())
    NCH = -(-NT // GCH)
    Kcum = np.zeros((NB, T), np.int64)
    acc = 0
    for b in range(NB):
        for t in range(T):
            Kcum[b, t] = acc
            acc += int(K[b, t])

    per_core = []
    for c in range(NC):
        ids = np.full((NT, 128), -1, dtype=np.int64)
        offs = np.full((NT, 128), -1, dtype=np.int64)
        for t in range(T):
            o, ecore, eslot = eslots[t]
            lo = np.searchsorted(ecore, c)
            hi = np.searchsorted(ecore, c + 1)
            sl = eslot[lo:hi]
            eid = o[lo:hi]
            bounds = np.searchsorted(sl, np.arange(NB + 1) * 128)
            for b in range(NB):
                s, e = bounds[b], bounds[b + 1]
                n = e - s
                if n == 0:
                    continue
                ti = int(Kcum[b, t])
                for k in range(int(K[b, t])):
                    a0, a1 = k * 128, min((k + 1) * 128, n)
                    m = a1 - a0
                    if m <= 0:
                        break
                    ids[ti + k, :m] = eid[s + a0:s + a1]
                    offs[ti + k, :m] = sl[s + a0:s + a1] & 127
        # eft tiles (transposed), chunk-major packing
        type_of_tile = np.zeros(NT, np.int64)
        for b in range(NB):
            for t in range(T):
                ti = int(Kcum[b, t])
                type_of_tile[ti:ti + int(K[b, t])] = t
        eft = np.zeros((NT, 128, FE), dtype=np.float32)
        for t in range(T):
            sel = np.nonzero(type_of_tile == t)[0]
            idsf = ids[sel]
            v = idsf >= 0
            ef = np.zeros((len(sel), 128, FE), np.float32)
            ef[v] = np.asarray(edge_feats[t])[idsf[v]]
            eft[sel] = ef
        eftT = eft.transpose(0, 2, 1)
        eftC = np.zeros((NCH, FE, GCH * 128), ml_dtypes.bfloat16)
        for ch in range(NCH):
            n_t = min(GCH, NT - ch * GCH)
            blk = eftT[ch * GCH: ch * GCH + n_t]
            eftC[ch, :, :n_t * 128] = blk.transpose(1, 0, 2).reshape(FE, n_t * 128)
        # one-hot (edge->slot) tiles in fp8, both orientations, chunk-packed
        oh = np.zeros((NT, 128, 128), np.float32)   # [tile, edge, slot]
        tt, ee = np.nonzero(offs >= 0)
        oh[tt, ee, offs[tt, ee]] = 1.0
        ohT = oh.transpose(0, 2, 1)                 # [tile, slot, edge]
        ohC = np.zeros((NCH, 128, GCH * 128), ml_dtypes.float8_e4m3)
        ohTC = np.zeros((NCH, 128, GCH * 128), ml_dtypes.float8_e4m3)
        for ch in range(NCH):
            n_t = min(GCH, NT - ch * GCH)
            blk = oh[ch * GCH: ch * GCH + n_t]
            ohC[ch, :, :n_t * 128] = _f8(
                blk.transpose(1, 0, 2).reshape(128, n_t * 128))
            blkT = ohT[ch * GCH: ch * GCH + n_t]
            ohTC[ch, :, :n_t * 128] = _f8(
                blkT.transpose(1, 0, 2).reshape(128, n_t * 128))
        per_core.append(dict(eft=eftC, ohc=ohC, ohtc=ohTC))
    return K, NT, per_core, slot_of_node


def make_feat_inputs(cfg: Cfg, features: np.ndarray, slot_of_node: np.ndarray):
    """Per-core packed featT over device slots: [NBJ, 128, JB*FKC*128]."""
    NC, R, F, JB, NB = cfg.ncores, cfg.R, cfg.F, cfg.JB, cfg.NB
    FKC = F // 128
    RS = cfg.RS
    NBJ = -(-NB // JB)
    feat_flat = np.asarray(features).reshape(-1, F)
    outs = []
    for c in range(NC):
        fs = np.zeros((RS, F), np.float32)
        lo, hi = c * R, min((c + 1) * R, feat_flat.shape[0])
        if hi > lo:
            fs[slot_of_node[c][:hi - lo]] = feat_flat[lo:hi]
        fc = fs.reshape(NB, 128, FKC, 128)
        ft = fc.transpose(0, 2, 3, 1)                # [NB, FKC, f, n]
        packed = np.zeros((NBJ, 128, JB * FKC * 128), ml_dtypes.bfloat16)
        for jc in range(NBJ):
            nb = min(JB, NB - jc * JB)
            blk = ft[jc * JB: jc * JB + nb]
            packed[jc, :, :nb * FKC * 128] = (
                blk.transpose(2, 0, 1, 3).reshape(128, nb * FKC * 128))
        outs.append(packed)
    return outs


def build_kernel(cfg: Cfg, K: np.ndarray, NT: int, dbg: bool = False, bench_iters: int = 0, ablate: str = ''):
    NBLK, T, U, FE, F = cfg.NBLK, cfg.T, cfg.U, cfg.FE, cfg.F
    GCH, JB, OB = cfg.GCH, cfg.JB, cfg.OB
    FKC = F // 128
    NCH = -(-NT // GCH)
    NBJ = -(-NBLK // JB)

    nc = bacc.Bacc("TRN2", target_bir_lowering=False, debug=False,
                   num_devices=cfg.ncores)

    featT = nc.dram_tensor("featT", [NBJ, 128, JB * FKC * 128], BF16,
                           kind="ExternalInput")
    wg = nc.dram_tensor("wg", [FKC, 128, U], BF16, kind="ExternalInput")
    wcat = nc.dram_tensor("wcat", [T, FE, 2 * U], BF16, kind="ExternalInput")
    btab = nc.dram_tensor("btab", [128, T * U], BF16, kind="ExternalInput")
    eft = nc.dram_tensor("eft", [NCH, FE, GCH * 128], BF16, kind="ExternalInput")
    ohc = nc.dram_tensor("ohc", [NCH, 128, GCH * 128], FP8, kind="ExternalInput")
    ohtc = nc.dram_tensor("ohtc", [NCH, 128, GCH * 128], FP8,
                          kind="ExternalInput")
    out = nc.dram_tensor("out", [NBLK, 128, U], BF16, kind="ExternalOutput")

    with TileContext(nc) as tc:
        with (
            tc.tile_pool(name="const", bufs=1) as constp,
            tc.tile_pool(name="ftile", bufs=(4 if "ftpre" in ablate else 3)) as ftp,
            tc.tile_pool(name="eftl", bufs=6) as eftp,
            tc.tile_pool(name="ohl", bufs=6) as ohp,
            tc.tile_pool(name="ohtl", bufs=6) as ohtp,
            tc.tile_pool(name="gate", bufs=10) as gatep,
            tc.tile_pool(name="valb", bufs=10) as valbp,
            tc.tile_pool(name="msgs", bufs=(22 if "lag18" in ablate else 12)) as msgsp,
            tc.tile_pool(name="outst", bufs=2) as outstp,
            tc.tile_pool(name="bank", bufs=6, space="PSUM") as bankp,
            tc.tile_pool(name="psout", bufs=2, space="PSUM") as psoutp,
        ):
            # ---- constants ----
            wg_sb = []
            for kc in range(FKC):
                w = constp.tile([128, U], BF16, tag=f"wg{kc}")
                nc.scalar.dma_start(out=w[:, :], in_=wg[kc, :, :])
                wg_sb.append(w)
            wcat_sb = []
            for t in range(T):
                w = constp.tile([FE, 2 * U], BF16, tag=f"wcat{t}")
                nc.scalar.dma_start(out=w[:, :], in_=wcat[t, :, :])
                wcat_sb.append(w)
            btab_sb = constp.tile([128, T * U], BF16, tag="btab")
            nc.scalar.dma_start(out=btab_sb[:, :], in_=btab[:, :])

            sgtab = constp.tile([128, NBLK * U], BF16, tag="sgtab")

            # ---- phase 1 emitters (interleaved into the phase-2 stream) ----
            import contextlib
            loop_cm = (tc.For_i(0, bench_iters, 1, hint_engines=(
                mybir.EngineType.PE, mybir.EngineType.DVE,
                mybir.EngineType.Activation, mybir.EngineType.Pool,
                mybir.EngineType.SP))
                if bench_iters else contextlib.nullcontext())
            loop_ctx = loop_cm.__enter__() if bench_iters else None

            ft_tiles = {}

            def ensure_ft(jc):
                if jc in ft_tiles or jc >= NBJ:
                    return
                nb = min(JB, NBLK - jc * JB)
                ft = ftp.tile([128, JB * FKC * 128], BF16, tag="ft",
                              name=f"ft{jc}")
                nc.scalar.dma_start(out=ft[:, :nb * FKC * 128],
                                    in_=featT[jc, :, :nb * FKC * 128])
                ft_tiles[jc] = ft

            def emit_p1_pair(j0):
                """sg for blocks j0, j0+1 (paired in one PSUM bank)."""
                jc = j0 // JB
                ensure_ft(jc)
                ensure_ft(jc + 1)
                if 'ftpre' in ablate:
                    ensure_ft(jc + 2)
                ft = ft_tiles[jc]
                jj = j0 - jc * JB
                npr = min(2, NBLK - j0)
                ps = bankp.tile([128, 2 * U], F32, tag="bk", name=f"p1_{j0}")
                for d in range(npr):
                    for kc in range(FKC):
                        o = ((jj + d) * FKC + kc) * 128
                        nc.tensor.matmul(ps[:, d * U:(d + 1) * U],
                                         ft[:, o:o + 128], wg_sb[kc][:, :],
                                         start=(kc == 0),
                                         stop=(kc == FKC - 1),
                                         skip_group_check=True)
                c0 = j0
                if (j0 // 2) % 2 == 0:
                    nc.scalar.copy(sgtab[:, c0 * U:(c0 + npr) * U],
                                   ps[:, :npr * U])
                else:
                    nc.vector.tensor_copy(sgtab[:, c0 * U:(c0 + npr) * U],
                                          ps[:, :npr * U])

            p1_state = dict(done=0)
            P1_AHEAD = 12    # keep sg filled this many blocks ahead of tiles

            def ensure_p1(upto):
                while p1_state['done'] < min(NBLK, upto):
                    emit_p1_pair(p1_state['done'])
                    p1_state['done'] += 2

            # ---- phase 2 ----
            eft_tiles = {}
            oh_tiles = {}
            oht_tiles = {}
            NBLK_eff = 0 if 'phase1' in ablate else NBLK

            def ensure_chunk(g):
                if g in eft_tiles:
                    return
                if 'nodma' in ablate and eft_tiles:
                    g0 = next(iter(eft_tiles))
                    eft_tiles[g] = eft_tiles[g0]
                    oh_tiles[g] = oh_tiles[g0]
                    oht_tiles[g] = oht_tiles[g0]
                    return
                t0 = g * GCH
                n_t = min(GCH, NT - t0)
                et = eftp.tile([FE, GCH * 128], BF16, tag="et", name=f"et{g}")
                nc.sync.dma_start(out=et[:, :n_t * 128],
                                  in_=eft[g, :, :n_t * 128])
                eft_tiles[g] = et
                ot = ohp.tile([128, GCH * 128], FP8, tag="oht", name=f"oh{g}")
                nc.scalar.dma_start(out=ot[:, :n_t * 128],
                                    in_=ohc[g, :, :n_t * 128])
                oh_tiles[g] = ot
                ott = ohtp.tile([128, GCH * 128], FP8, tag="ohtt",
                                name=f"oht{g}")
                nc.gpsimd.dma_start(out=ott[:, :n_t * 128],
                                    in_=ohtc[g, :, :n_t * 128])
                oht_tiles[g] = ott

            LAG = 18 if "lag18" in ablate else 12
            pending = []          # (oh_ap, msgs_ap, pso_region, start, stop, flush)
            state = dict(pso=None, pso_g0=None)

            def emit_scatter(ent):
                oh_ap, msgs_ap, pso_, st_, sp_, flush = ent
                nc.tensor.matmul(pso_, oh_ap, msgs_ap, start=st_, stop=sp_,
                                 skip_group_check=True)
                if flush is not None:
                    flush()

            # flat tile schedule: (block, type, first/last in block)
            sched = []
            for b in range(NBLK_eff):
                ntile_b = int(K[b].sum())
                done = 0
                for t in range(T):
                    for k in range(int(K[b, t])):
                        sched.append((b, t, done == 0, done == ntile_b - 1))
                        done += 1

            ntiles_of = [int(K[b].sum()) for b in range(NBLK)]
            flush_owner = {}
            for g0 in range(0, NBLK, OB):
                grp = [b for b in range(g0, min(g0 + OB, NBLK))]
                live = [b for b in grp if ntiles_of[b] > 0]
                flush_owner[g0] = live[-1] if live else None

            def group_prolog(g0):
                if state['pso_g0'] == g0:
                    return
                state['pso_g0'] = g0
                nb = min(OB, NBLK - g0)
                state['pso'] = psoutp.tile([128, OB * U], F32, tag="pso",
                                           name=f"pso{g0}")
                for bb in range(g0, g0 + nb):
                    if ntiles_of[bb] == 0:
                        nc.vector.memset(
                            state['pso'][:, (bb - g0) * U:(bb - g0 + 1) * U],
                            0.0)

            def make_flush(g0, pso):
                def flush():
                    nb = min(OB, NBLK - g0)
                    ost = outstp.tile([128, OB * U], BF16, tag="ost",
                                      name=f"ost{g0}")
                    nc.scalar.copy(ost[:, :nb * U], pso[:, :nb * U])
                    oq = nc.gpsimd if 'outswdge' in ablate else nc.sync
                    oq.dma_start(
                        out=out[g0:g0 + nb, :, :].rearrange("j p u -> p j u"),
                        in_=ost[:, :nb * U].rearrange("p (j u) -> p j u", u=U))
                return flush

            if 'noact' in ablate:
                dummy_msgs = constp.tile([128, U], BF16, tag="dummy")
                nc.vector.memset(dummy_msgs[:, :], 0.25)

            ensure_p1(P1_AHEAD)
            for ii, (b, t, first, last) in enumerate(sched):
                ensure_p1(b + P1_AHEAD)
                g, s = divmod(ii, GCH)
                ensure_chunk(g)
                if ii % GCH == 0:
                    for gg in (g + 1, g + 2, g + 3):
                        if gg * GCH < NT:
                            ensure_chunk(gg)
                ef = eft_tiles[g][:, s * 128:(s + 1) * 128]
                oht_ap = oht_tiles[g][:, s * 128:(s + 1) * 128]
                oh_ap = oh_tiles[g][:, s * 128:(s + 1) * 128]
                gsl = slice(0, U)
                vsl = slice(U, 2 * U)
                bank = bankp.tile([128, 2 * U], F32, tag="bk")
                nc.tensor.matmul(bank[:, :], ef, wcat_sb[t][:, :],
                                 start=True, stop=('nogather' in ablate),
                                 skip_group_check=True)
                if 'nogather' not in ablate:
                    nc.tensor.matmul(bank[:, gsl], oht_ap,
                                     sgtab[:, b * U:(b + 1) * U],
                                     start=False, stop=True,
                                     skip_group_check=True)
                if 'noact' in ablate:
                    # timing diagnostic: decouple PE from the ACT/DVE chain
                    msgs = dummy_msgs
                else:
                    gate = gatep.tile([128, U], BF16)
                    nc.scalar.activation(gate[:, :], bank[:, gsl], AF.Sigmoid)
                    # bias add on DVE (off the PE): val+b -> bf16, then a
                    # cheap bf16x bf16 multiply
                    valb = valbp.tile([128, U], BF16)
                    nc.vector.tensor_tensor(valb[:, :], bank[:, vsl],
                                            btab_sb[:, t * U:(t + 1) * U],
                                            ALU.add)
                    msgs = msgsp.tile([128, U], BF16)
                    nc.vector.tensor_tensor(msgs[:, :], gate[:, :],
                                            valb[:, :], ALU.mult)
                g0 = (b // OB) * OB
                if first:
                    group_prolog(g0)
                if 'noscatter' not in ablate:
                    pso_region = state['pso'][:, (b - g0) * U:(b - g0 + 1) * U]
                    ent = [oh_ap, msgs[:, :], pso_region, first, last, None]
                    if last and flush_owner[g0] == b:
                        ent[5] = make_flush(g0, state['pso'])
                    pending.append(ent)
                    if len(pending) > LAG:
                        emit_scatter(pending.pop(0))
            ensure_p1(NBLK)     # sg for any trailing edge-less blocks
            for ent in pending:
                emit_scatter(ent)
            # groups consisting entirely of empty blocks
            if NBLK_eff:
                for g0 in range(0, NBLK, OB):
                    if flush_owner[g0] is None:
                        group_prolog(g0)
                        make_flush(g0, state['pso'])()
            if bench_iters:
                loop_cm.__exit__(None, None, None)
    nc.compile()
    return nc


def make_const_inputs(cfg: Cfg, W_gate, W_gate_e, W_dense, b_dense):
    FKC = cfg.F // 128
    T, U = cfg.T, cfg.U
    btab_np = np.broadcast_to(
        np.asarray(b_dense, np.float32).reshape(1, T * U), (128, T * U))
    return dict(
        wg=_bf(np.ascontiguousarray(
            np.asarray(W_gate, np.float32).reshape(FKC, 128, cfg.U))),
        wcat=_bf(np.concatenate([np.asarray(W_gate_e, np.float32),
                                 np.asarray(W_dense, np.float32)], axis=2)),
        btab=_bf(btab_np.copy()),
    )


def make_in_maps(cfg: Cfg, inputs):
    K, NT, per_core, slot_of_node = preprocess(
        cfg, inputs['edge_idx'], inputs['edge_feats'])
    feat_in = make_feat_inputs(cfg, inputs['features'], slot_of_node)
    const_in = make_const_inputs(cfg, inputs['W_gate'], inputs['W_gate_e'],
                                 inputs['W_dense'], inputs['b_dense'])
    in_maps = []
    for c in range(cfg.ncores):
        m = dict(const_in)
        m['featT'] = feat_in[c]
        m.update(per_core[c])
        in_maps.append(m)
    return K, NT, in_maps, slot_of_node


def extract_output(cfg: Cfg, results, slot_of_node):
    out_full = np.zeros((cfg.ncores * cfg.R, cfg.U), np.float32)
    for c in range(cfg.ncores):
        dev = np.asarray(results[c]['out']).astype(np.float32)
        dev = dev.reshape(cfg.RS, cfg.U)
        out_full[c * cfg.R:(c + 1) * cfg.R] = dev[slot_of_node[c]]
    return out_full[:cfg.BN]


def run_full(cfg: Cfg, inputs, run_fn):
    K, NT, in_maps, slot_of_node = make_in_maps(cfg, inputs)
    nc = build_kernel(cfg, K, NT)
    results = run_fn(nc, in_maps)
    return extract_output(cfg, results, slot_of_node)


# ============================================================================
# Self-contained entry point (harness contract):
#   kernel(**inputs) takes the FULL unsharded inputs and returns the FULL
#   output [2, 50000, 256] float32. Internally: node-shard across the 8
#   NeuronCores (no collectives needed since gather idx == scatter idx per
#   edge), compile one SPMD Bass program, run via run_bass_kernel_spmd.
# ============================================================================
from concourse.bass_utils import run_bass_kernel_spmd

_CACHE = {}


def kernel(features, edge_idx, edge_feats, W_gate, W_gate_e, W_dense, b_dense):
    features = np.asarray(features)
    edge_idx = np.asarray(edge_idx)
    edge_feats = np.asarray(edge_feats)
    B, N, F = features.shape
    BN = B * N
    cfg = Cfg(ncores=8, R=-(-BN // (8 * 128)) * 128, F=F,
              U=np.asarray(W_gate).shape[1], FE=edge_feats.shape[2],
              T=edge_feats.shape[0], BN=BN)
    cfg.NB = -(-cfg.R // 128) + 34      # packing slack (~35% spare slots)

    inputs = dict(features=features, edge_idx=edge_idx, edge_feats=edge_feats,
                  W_gate=W_gate, W_gate_e=W_gate_e, W_dense=W_dense,
                  b_dense=b_dense)
    K, NT, in_maps, slot_of_node = make_in_maps(cfg, inputs)

    key = (cfg.R, cfg.NB, cfg.F, cfg.U, cfg.FE, cfg.T, NT, K.tobytes())
    nc = _CACHE.get(key)
    if nc is None:
        nc = build_kernel(cfg, K, NT)
        _CACHE[key] = nc

    res = run_bass_kernel_spmd(nc, in_maps, core_ids=list(range(cfg.ncores)))
    out = extract_output(cfg, res.results, slot_of_node)
    return out.reshape(B, N, cfg.U).astype(np.float32)
